# revision 21
# baseline (speedup 1.0000x reference)
"""CQT magnitude kernel for Trainium2 (8 NeuronCores, Bass/Tile).

Strategy (v3)
-------------
C[k, n] = sum_l xpad[n*HOP + l] * kernel[k, l], tiled over 128-wide
l-chunks.  Core q owns chunks c = 8s+q (m0 = bins 0-127) and
c = 247+8s+q (m1 = bins 128-251); host sums the 8 per-core partials.

Key points vs v2 (74.8us -> target ~31us):
 * Contiguous rhs streams.  Core q only ever touches xpad chunks of a
   single residue class mod 4 (j = q mod 4 for m0, q+3 mod 4 for m1), so
   xi is packed per-class as [128, v, 4 tracks] with v = signal chunk/4.
   The matmul rhs walk [ [4,F], [1,4] ] is then fully sequential in
   SBUF.  Measured: strided rhs streams at 0.90 ns/col, contiguous at
   0.427 ns/col (1 col/cycle @ 2.4 GHz) -- a 2.1x PE speedup.
 * re/im interleaved weight columns: col 2t = kr[bin t], 2t+1 = ki.
   For chunks with M<=64 active bins one matmul computes both parts
   (row 2t = re, 2t+1 = im), halving streamed columns there.  Bins are
   split lo (0-63) / hi (64-127) per half so PSUM row meanings stay
   uniform across chunks; hi entries exist only where M>64.
 * 8 PSUM accumulators: (m0,m1) x (lo,hi) x (frame half 0-64 / 65-128),
   each [128, <=260] f32 (2KB bank limit forces the frame split).
 * Banks are initialized by zero-weight matmuls (start=True, full
   rectangle) during the DMA wait -- they double as PE p-state warmup.
   All real entries accumulate with start=False in any order, so the
   short, LDWEIGHTS-bound edge entries run first while the PE clock
   still ramps.
 * Flush pipelining: m1 pass runs early and its banks flush under the
   long m0 pass; m0-hi banks flush under the m0 lo-only tail; only the
   m0-lo flush remains in the tail.  Output staged to bf16 (halves
   DMA; partial sums re-summed on host in f32, ~0.1% error).
"""

import numpy as np

# ---- problem constants (hardcoded per contract) ----
SR = 44100
BPO = 36
KBINS = 252
FMIN = 32.70319566257483
QF = 1.0 / (2.0 ** (1.0 / BPO) - 1.0)
SR_B, SR_TR, SR_T = 2, 2, 65536
NTRACKS = SR_B * SR_TR            # 4
L = 69376                          # filterbank window length
HOP = 512
PCH = 128
NCH = L // PCH                     # 542 l-chunks
NF = 1 + SR_T // HOP               # 129 frames
NCORES = 8
M1C0 = 247                         # first m1 chunk
NS0 = 68                           # m0 slots per core
NS1 = 6                            # m1 slots per core (48 chunks)
J_VALID_LO, J_VALID_HI = 271, 782  # nonzero xpad chunk range (inclusive)
XPAD_CH = 1056
FH = 65                            # frame-half boundary: fh0=[0,64], fh1=[65,128]
VA0, VA = 64, 134                  # xiA v-window (v = (j - q)/4)
VB0, VB = 64, 133                  # xiB v-window (v = (j - q - 3)/4)
N_PRE = 10                         # zero-weight PE ramp-hold matmuls
S0_INIT = 33                       # m0 init slot (full frame coverage)
S1_INIT = 2                        # m1 init slot (forced full coverage)

# ---- slot tables ----


def _build_tables():
    freqs = FMIN * 2.0 ** (np.arange(KBINS) / BPO)
    lens = QF * SR / freqs
    lo = np.floor((L // 2 - lens / 2) / PCH).astype(int)
    hi = np.ceil((L // 2 + lens / 2) / PCH).astype(int)
    m0c = np.zeros(NCH + 8, int)
    m1c = np.zeros(NCH + 8, int)
    for k in range(128):
        m0c[lo[k] : hi[k]] = np.maximum(m0c[lo[k] : hi[k]], k + 1)
    for k in range(128, KBINS):
        m1c[lo[k] : hi[k]] = np.maximum(m1c[lo[k] : hi[k]], k - 127)
    m0s = [max(m0c[8 * s + q] for q in range(8)) for s in range(NS0)]
    m1s = [max(m1c[M1C0 + 8 * s + q] for q in range(8)) for s in range(NS1)]

    def nrng(cl, ch):
        return max(0, -(-(J_VALID_LO - ch) // 4)), min(
            NF - 1, (J_VALID_HI - cl) // 4
        )

    f0 = [nrng(8 * s, 8 * s + 7) for s in range(NS0)]
    f1 = [nrng(M1C0 + 8 * s, M1C0 + 8 * s + 7) for s in range(NS1)]

    # emission order: INIT0 (m0 full-coverage start=True, runs during the
    # PE clock ramp) | E (short-F edge slots, LDWEIGHTS-bound, ramp-
    # insensitive) | A (m0 rest, hi-carrying slots first) | INIT1+B (m1
    # last: small pass that hides the A-lo flush; tail = B-lo only)
    e_slots = []
    for i in range(9):
        e_slots.append(i)
        e_slots.append(67 - i)
    hi_slots = [s for s in range(NS0) if m0s[s] > 64 and s != S0_INIT]
    a_rest = [s for s in range(9, 59) if s not in hi_slots and s != S0_INIT]
    b_order = [3, 1, 4, 0, 5]

    # entry: (m, s, kind, cols, n0, n1)   kind 0=lo 1=hi
    entries = [
        (0, S0_INIT, 0, 128, 0, NF - 1),
        (0, S0_INIT, 1, 128, 0, NF - 1),
    ]
    for s in e_slots:
        entries.append((0, s, 0, 2 * min(m0s[s], 64), f0[s][0], f0[s][1]))
    for s in hi_slots:
        entries.append((0, s, 0, 2 * min(m0s[s], 64), f0[s][0], f0[s][1]))
        entries.append((0, s, 1, 2 * (m0s[s] - 64), f0[s][0], f0[s][1]))
    for s in a_rest:
        entries.append((0, s, 0, 2 * min(m0s[s], 64), f0[s][0], f0[s][1]))
    ib0 = len(entries)
    entries.append((1, S1_INIT, 0, 128, 0, NF - 1))
    entries.append((1, S1_INIT, 1, 128, 0, NF - 1))  # cols padded past 251
    for s in b_order:
        entries.append((1, s, 0, 2 * min(m1s[s], 64), f1[s][0], f1[s][1]))
        if m1s[s] > 64:
            entries.append((1, s, 1, 2 * (m1s[s] - 64), f1[s][0], f1[s][1]))

    n_e = len(e_slots)
    a_hi_end = 2 + n_e + 2 * len(hi_slots)
    starts = (0, 1, ib0, ib0 + 1)
    bhi_end = ib0 + 4        # after INIT1 lo/hi + s3 lo/hi
    # kt DMA groups: INIT0 | E | A thirds | INIT1+B
    offs = np.cumsum([0] + [e[3] for e in entries])
    a0 = 2 + n_e
    rest = int(offs[ib0] - offs[a0])
    t1 = int(offs[a0]) + rest // 3
    t2 = int(offs[a0]) + 2 * rest // 3
    g2 = int(np.searchsorted(offs, t1))
    g3 = int(np.searchsorted(offs, t2))
    gsplits = [0, 2, a0, g2, g3, ib0, len(entries)]
    return entries, offs, gsplits, (starts, a_hi_end, ib0, bhi_end)


_ENTRIES, _KTOFF, _GSPL, (_STARTS, _A_HI_END, _B0, _BHI_END) = _build_tables()
_NG = len(_GSPL) - 1
_GCOLS = [int(_KTOFF[_GSPL[g + 1]] - _KTOFF[_GSPL[g]]) for g in range(_NG)]

# bank ids: 0=A_lo0 1=A_lo1 2=A_hi0 3=A_hi1 4=B_lo0 5=B_lo1 6=B_hi0 7=B_hi1
_BANK_COLS = [260, 256, 260, 256, 260, 256, 260, 256]


def _bank_of(m, kind, fh):
    return (4 if m == 1 else 0) + 2 * kind + fh


def _fh_windows(n0, n1):
    """Split [n0, n1] at the frame-half boundary; yields (fh, a, b)."""
    out = []
    if n0 < FH:
        out.append((0, n0, min(n1, FH - 1)))
    if n1 >= FH:
        out.append((1, max(n0, FH), n1))
    return out


def _last_writer_per_bank():
    last = {}
    for ei, (m, s, kind, cols, n0, n1) in enumerate(_ENTRIES):
        for fh, a, b in _fh_windows(n0, n1):
            last[_bank_of(m, kind, fh)] = ei
    return last


_LAST = _last_writer_per_bank()

_PROG = None


def _build_program():
    import concourse.bass as bass
    import concourse.mybir as mybir
    from concourse import bacc
    from concourse.tile import TileContext

    f32 = mybir.dt.float32
    bf16 = mybir.dt.bfloat16

    nc = bacc.Bacc(None, name="cqt_spmd3")
    kt_d = [
        nc.dram_tensor(f"kt{g}", [128, _GCOLS[g]], bf16, kind="ExternalInput")
        for g in range(_NG)
    ]
    xa_d = nc.dram_tensor("xa", [128, VA, 4], bf16, kind="ExternalInput")
    xb_d = nc.dram_tensor("xb", [128, VB, 4], bf16, kind="ExternalInput")
    OUTC = sum(_BANK_COLS)
    out_d = nc.dram_tensor("out", [128, OUTC], bf16, kind="ExternalOutput")

    with TileContext(nc) as tc:
        with (
            tc.tile_pool(name="xp", bufs=1) as xp,
            tc.tile_pool(name="ktp", bufs=1) as ktp,
            tc.tile_pool(name="wp", bufs=1) as wp,
            tc.tile_pool(name="accp", bufs=1, space="PSUM") as accp,
        ):
            wtile = wp.tile([128, 128], bf16, tag="wt", name="wt")
            nc.vector.memset(wtile, 0.0)

            xa_t = xp.tile([128, VA, 4], bf16, tag="xa", name="xa")
            xb_t = xp.tile([128, VB, 4], bf16, tag="xb", name="xb")
            kt_t = [
                ktp.tile([128, _GCOLS[g]], bf16, tag=f"kt{g}", name=f"kt{g}")
                for g in range(_NG)
            ]
            # DMA issue split across engines; critical (kt-init0, xa, ktE)
            # first, m1 inputs (xb, kt5) last
            nc.scalar.dma_start(out=kt_t[0], in_=kt_d[0][:, :])
            nc.gpsimd.dma_start(out=xa_t, in_=xa_d[:, :, :])
            nc.scalar.dma_start(out=kt_t[1], in_=kt_d[1][:, :])
            nc.gpsimd.dma_start(out=kt_t[2], in_=kt_d[2][:, :])
            nc.scalar.dma_start(out=kt_t[3], in_=kt_d[3][:, :])
            nc.gpsimd.dma_start(out=kt_t[4], in_=kt_d[4][:, :])
            nc.scalar.dma_start(out=xb_t, in_=xb_d[:, :, :])
            nc.gpsimd.dma_start(out=kt_t[5], in_=kt_d[5][:, :])

            # full 2KB banks so no tile ever crosses a PSUM bank boundary
            accs = [
                accp.tile([128, 512], f32, tag=f"acc{b}", name=f"acc{b}")
                for b in range(8)
            ]

            # zero-weight ramp-hold matmuls (no data deps); real INIT
            # entries re-init bank 0 with start=True afterwards
            def zrhs(cols):
                return bass.AP(
                    tensor=wtile.tensor,
                    offset=wtile.offset,
                    ap=[wtile.ap[0], [0, cols // 4], [1, 4]],
                )

            for _ in range(N_PRE):
                nc.tensor.matmul(
                    accs[0][:128, :260], wtile[:, :128], zrhs(260),
                    start=True, stop=True,
                )

            st = wp.tile([128, OUTC], bf16, tag="st", name="st")
            boff = np.cumsum([0] + _BANK_COLS)

            def flush(banks, eng_cycle, dma_engs=None):
                for i, b in enumerate(banks):
                    eng = eng_cycle[i % len(eng_cycle)]
                    src = accs[b][:128, : _BANK_COLS[b]]
                    dst = st[:, int(boff[b]) : int(boff[b + 1])]
                    if hasattr(eng, "tensor_copy"):
                        eng.tensor_copy(dst, src)
                    else:
                        eng.copy(dst, src)
                lo = int(boff[banks[0]])
                hi = int(boff[banks[-1] + 1])
                dma_engs = dma_engs or [nc.gpsimd]
                n = len(dma_engs)
                cuts = [lo + (hi - lo) * i // n for i in range(n + 1)]
                for i, eng in enumerate(dma_engs):
                    eng.dma_start(
                        out=out_d[:, cuts[i] : cuts[i + 1]],
                        in_=st[:, cuts[i] : cuts[i + 1]],
                    )

            g = 0
            for ei, (m, s, kind, cols, n0, n1) in enumerate(_ENTRIES):
                while ei >= _GSPL[g + 1]:
                    g += 1
                off = int(_KTOFF[ei] - _KTOFF[_GSPL[g]])
                lhsT = kt_t[g][:, off : off + cols]
                xi = xb_t if m == 1 else xa_t
                vbase = (61 + 2 * s - VB0) if m == 1 else (2 * s - VA0)
                for fh, a, b in _fh_windows(n0, n1):
                    F = b - a + 1
                    rhs = bass.AP(
                        tensor=xi.tensor,
                        offset=xi.offset + (vbase + a) * 4,
                        ap=[xi.ap[0], [4, F], [1, 4]],
                    )
                    bk = _bank_of(m, kind, fh)
                    fb = 0 if fh == 0 else FH
                    out = accs[bk][:cols, 4 * (a - fb) : 4 * (b + 1 - fb)]
                    nc.tensor.matmul(
                        out, lhsT, rhs,
                        start=(ei in _STARTS), stop=(_LAST[bk] == ei),
                    )
                if ei == _A_HI_END - 1:     # end of m0 hi entries
                    flush([2, 3], [nc.vector, nc.scalar])
                if ei == _B0 - 1:           # end of m0: flush lo under B pass
                    flush([0, 1], [nc.vector, nc.scalar],
                          [nc.gpsimd, nc.sync])
                if ei == _BHI_END - 1:      # m1 hi banks complete
                    flush([6, 7], [nc.vector, nc.scalar])
            # tail flush: B-lo banks, out-DMA in 3 pieces on 3 queues
            for i, b in enumerate([4, 5]):
                eng = [nc.vector, nc.scalar][i]
                src = accs[b][:128, : _BANK_COLS[b]]
                dst = st[:, int(boff[b]) : int(boff[b + 1])]
                if hasattr(eng, "tensor_copy"):
                    eng.tensor_copy(dst, src)
                else:
                    eng.copy(dst, src)
            lo4, hi4 = int(boff[4]), int(boff[6])
            w = hi4 - lo4
            cuts = [lo4, lo4 + w // 3, lo4 + 2 * w // 3, hi4]
            for i, eng in enumerate([nc.gpsimd, nc.scalar, nc.sync]):
                eng.dma_start(
                    out=out_d[:, cuts[i] : cuts[i + 1]],
                    in_=st[:, cuts[i] : cuts[i + 1]],
                )
    nc.finalize()
    _dedupe_ldweights(nc)
    return nc


def _dedupe_ldweights(nc):
    """Drop back-to-back InstLdweights with identical weights APs."""
    for fn in nc.m.functions:
        for bb in fn.blocks:
            insts = list(bb.instructions)
            keep = []
            prev_key = None
            for inst in insts:
                if type(inst).__name__ == 'InstLdweights':
                    key = str(inst.ins[0])
                    si = inst.sync_info
                    clean = not si or (
                        len(si.on_wait) == 0 and len(si.on_update) == 0
                    )
                    if key == prev_key and clean:
                        continue
                    prev_key = key
                keep.append(inst)
            if len(keep) != len(insts):
                bb.instructions = keep


def _pack_inputs(x, kr, ki):
    import ml_dtypes

    bf16 = ml_dtypes.bfloat16
    xf = np.ascontiguousarray(
        np.asarray(x, dtype=np.float32).reshape(NTRACKS, SR_T)
    )
    kr = np.asarray(kr, dtype=np.float32)
    ki = np.asarray(ki, dtype=np.float32)

    xpad = np.zeros((NTRACKS, XPAD_CH * PCH), np.float32)
    xpad[:, L // 2 : L // 2 + SR_T] = xf
    xch = xpad.reshape(NTRACKS, XPAD_CH, PCH)      # [t, j, p]

    in_maps = []
    for q in range(NCORES):
        ja = q + 4 * (VA0 + np.arange(VA))
        jb = (q + 3) + 4 * (VB0 + np.arange(VB))
        xa = np.ascontiguousarray(
            xch[:, ja, :].transpose(2, 1, 0).astype(bf16)
        )  # [128, VA, 4]
        xb = np.ascontiguousarray(
            xch[:, jb, :].transpose(2, 1, 0).astype(bf16)
        )
        kt = np.zeros((128, int(_KTOFF[-1])), np.float32)
        for ei, (m, s, kind, cols, n0, n1) in enumerate(_ENTRIES):
            c = (M1C0 + 8 * s + q) if m == 1 else (8 * s + q)
            if m == 0 and c >= NCH:
                continue  # chunks 542/543 on cores 6-7: zero weights
            binoff = (128 if m == 1 else 0) + (64 if kind == 1 else 0)
            nb = min(cols // 2, KBINS - binoff)   # init m1-hi pads past 251
            off = int(_KTOFF[ei])
            blk = kt[:, off : off + cols]
            sl = slice(c * PCH, (c + 1) * PCH)
            blk[:, 0 : 2 * nb : 2] = kr[binoff : binoff + nb, sl].T
            blk[:, 1 : 2 * nb : 2] = ki[binoff : binoff + nb, sl].T
        ktb = kt.astype(bf16)
        im = {
            f"kt{g}": np.ascontiguousarray(
                ktb[:, int(_KTOFF[_GSPL[g]]) : int(_KTOFF[_GSPL[g + 1]])]
            )
            for g in range(_NG)
        }
        im["xa"] = xa
        im["xb"] = xb
        in_maps.append(im)
    return in_maps


def _combine(outs):
    boff = np.cumsum([0] + _BANK_COLS)
    re_acc = np.zeros((KBINS, NF, NTRACKS), np.float32)
    im_acc = np.zeros((KBINS, NF, NTRACKS), np.float32)
    # bank -> (bin base, frame base)
    meta = {0: (0, 0), 1: (0, FH), 2: (64, 0), 3: (64, FH),
            4: (128, 0), 5: (128, FH), 6: (192, 0), 7: (192, FH)}
    for q in range(NCORES):
        o = np.asarray(outs[q]).astype(np.float32)
        for b in range(8):
            kb, fb = meta[b]
            nfr = (_BANK_COLS[b]) // 4
            nbins = min(64, KBINS - kb)
            blk = o[: 2 * nbins, int(boff[b]) : int(boff[b + 1])]
            blk = blk.reshape(2 * nbins, nfr, 4)
            re_acc[kb : kb + nbins, fb : fb + nfr] += blk[0::2]
            im_acc[kb : kb + nbins, fb : fb + nfr] += blk[1::2]
    y = np.sqrt(re_acc**2 + im_acc**2)  # [252, 129, 4]
    # output (B, K, NF, Tr): track t = b*SR_TR + tr
    y = y.reshape(KBINS, NF, SR_B, SR_TR)
    return np.ascontiguousarray(y.transpose(2, 0, 1, 3))


def kernel(x, kr, ki):
    global _PROG
    from concourse.bass_utils import run_bass_kernel_spmd

    if _PROG is None:
        _PROG = _build_program()
    in_maps = _pack_inputs(x, kr, ki)
    res = run_bass_kernel_spmd(_PROG, in_maps, core_ids=list(range(NCORES)))
    outs = [res.results[q]["out"] for q in range(NCORES)]
    return _combine(outs)
